# revision 7
# baseline (speedup 1.0000x reference)
"""AMMLinear (vq_codebook) forward kernel for 8 TRN2 NeuronCores.

Key algebraic fact: the reference's straight-through estimator
    output = real - stop_grad(real - quantized)
is numerically exactly `quantized_output + bias`, so the forward value needs
only:  argmin-distance one-hot  @  fake-quantized lut  + bias.

Distribution: pure data-parallel over the 8192 tokens (1024/core) with ZERO
collectives -- cores run fully independently (no barrier / AllReduce /
AllGather latency, immune to core start skew).

The quantized lut q = clip(round(lut/scale), -127, 127) is x-independent
(derived from centroids/weight only), so it is computed EXACTLY on host
(float64, matching the oracle) and shipped to the cores -- no on-device lut
matmuls or quantize epilogue at all.

The gather  out[n,o] = sum_c q[c, argmin_k dist, o]  runs as fp8e4m3
DoubleRow matmuls: q splits exactly as q = qa16 + qb with qa16 = 16*round(
q/16) (multiples of 16, |.|<=128) and qb = q - qa16 (ints, |.|<=8) -- both
exact in e4m3, as are the 0/1 one-hot weights.  DoubleRow packs TWO
codebook-groups (2 x 8 codebooks x 16 centroids = 256) into one matmul
contraction, so each 512-wide output chunk accumulates (4 group-pairs) x
(a,b passes) = 8 matmuls -- the same column count as fp16 (exact int8 is
2x fp8 information; DR's doubled contraction exactly pays for the a/b
split) but with HALF the weight loads, each reused across 4 matmuls.
The one-hot is the stationary operand (reused across all 4096 out cols),
token-major: psum[tok128, ocols] so the PSUM drain is a single dtype-convert
copy (the psum holds exact integer sums |.|<=8128; fp16 rounding of those is
<= 2^-12 relative) alternating DVE/Act, and the out DMA is fp16.  The
x-independent  out * scale + bias  epilogue runs on host in fp32.

Scores are ONE fp32 matmul pass per (tile, group) -- fp32 matmuls run at 4
cycles/row so a 128-col score matmul costs the same ~213ns as the fp16
3-pass hi/lo scheme's three 128-col passes combined, with exact-fp32
argmins and a third of the weight loads.  The c2 row-pair init stays two
fp16 K=2 matmuls (hi+lo summed in-psum).  The argmax chain is split DVE
(psum max + is_equal, frees the psum bank early) -> GpSimd (first-hit
encode + reduce) so neither engine paces the PE.

Per-core pipeline: score tiles -> split argmax chain -> PE transpose ->
one-hot expand (broadcast DMA + is_equal to fp8) -> gather units (t,
o-quarter): 16 DoubleRow matmuls into a [128,1024] psum, convert-drain,
fp16 DMA out.  Host concatenates core shards and applies scale+bias.
"""

import numpy as np

N_TOKENS = 8192
IN_FEAT = 1024
C = 64   # codebooks
KC = 16  # centroids per codebook
S = 16   # subvector length
O = 4096  # out features
NCORES = 8
NLOC = N_TOKENS // NCORES  # 1024 tokens per core
G = 8    # groups of 8 codebooks -> 128-row contraction
GP = 4   # group-pairs (DoubleRow: 2 groups = 256-row contraction)
TT = NLOC // 128  # 8 token tiles
NQ = 4   # o-quarters of 1024 cols (one gather unit each)

_CACHED = {}


def build_nc():
    import concourse.bacc as bacc
    import concourse.mybir as mybir
    import concourse.tile as tile
    from contextlib import ExitStack

    f32 = mybir.dt.float32
    f16 = mybir.dt.float16
    f8 = mybir.dt.float8e4
    AO = mybir.AluOpType
    AF = mybir.ActivationFunctionType
    DR = mybir.MatmulPerfMode.DoubleRow
    X = mybir.AxisListType.X

    nc = bacc.Bacc(
        "TRN2", target_bir_lowering=False, debug=False, num_devices=NCORES
    )

    xf_d = nc.dram_tensor("xf", [128, TT, G, 128], f32, kind="ExternalInput")
    q8a_d = nc.dram_tensor("q8a", [128, NQ, G, 1024], f8, kind="ExternalInput")
    q8b_d = nc.dram_tensor("q8b", [128, NQ, G, 1024], f8, kind="ExternalInput")
    bdf_d = nc.dram_tensor("bdf", [128, G, 128], f32, kind="ExternalInput")
    nc2hl_d = nc.dram_tensor("nc2hl", [2, 1024], f16, kind="ExternalInput")
    or2_d = nc.dram_tensor("or2", [2, 128], f16, kind="ExternalInput")
    kiota_d = nc.dram_tensor("kiota", [128, 1], f16, kind="ExternalInput")
    ioneg_d = nc.dram_tensor("ioneg", [128, 1024], f16, kind="ExternalInput")
    idb_d = nc.dram_tensor("idb", [128, 128], f16, kind="ExternalInput")
    out_d = nc.dram_tensor("out", [NLOC, O], f16, kind="ExternalOutput")

    with ExitStack() as ctx:
        tc = ctx.enter_context(tile.TileContext(nc))
        sb = ctx.enter_context(tc.tile_pool(name="sb", bufs=1))
        sbx = ctx.enter_context(tc.tile_pool(name="sbx", bufs=3))
        sbm = ctx.enter_context(tc.tile_pool(name="sbm", bufs=2))
        sbo = ctx.enter_context(tc.tile_pool(name="sbo", bufs=3))
        sbi = ctx.enter_context(tc.tile_pool(name="sbi", bufs=4))
        psS = ctx.enter_context(tc.tile_pool(name="psS", bufs=3, space="PSUM"))
        psT = ctx.enter_context(tc.tile_pool(name="psT", bufs=1, space="PSUM"))

        # ---------- persistent SBUF ----------
        bdf_sb = sb.tile([128, G, 128], f32)
        nc2hl_sb = sb.tile([2, 1024], f16)
        or2_sb = sb.tile([2, 128], f16)
        kiota_sb = sb.tile([128, 1], f16)
        ioneg_sb = sb.tile([128, 1024], f16)
        idb_sb = sb.tile([128, 128], f16)
        q8a_sb = sb.tile([128, NQ, G, 1024], f8)
        q8b_sb = sb.tile([128, NQ, G, 1024], f8)
        oh8_sb = sb.tile([128, G, NLOC], f8)
        idxT_sb = sb.tile([64, NLOC], f16)

        # ---------- const + input DMAs ----------
        # small consts first so the first score matmuls aren't blocked
        nc.gpsimd.dma_start(nc2hl_sb[:], nc2hl_d[:])
        nc.gpsimd.dma_start(or2_sb[:], or2_d[:])
        nc.gpsimd.dma_start(kiota_sb[:], kiota_d[:])
        nc.gpsimd.dma_start(ioneg_sb[:], ioneg_d[:])
        nc.gpsimd.dma_start(idb_sb[:], idb_d[:])
        nc.gpsimd.dma_start(bdf_sb[:], bdf_d[:])

        # fp32 x token tiles, alternating scalar/sync DMA queues
        xf_tiles = [
            sbx.tile([128, G, 128], f32, tag="xf", name=f"xf{t}")
            for t in range(TT)
        ]
        for t in range(TT):
            eng = nc.scalar if t % 2 == 0 else nc.sync
            eng.dma_start(xf_tiles[t][:], xf_d[:, t])

        # quantized-lut pieces on the sync queue, one per (quarter, a/b)
        for qtr in range(NQ):
            nc.sync.dma_start(q8a_sb[:, qtr], q8a_d[:, qtr])
            nc.sync.dma_start(q8b_sb[:, qtr], q8b_d[:, qtr])

        # ---------- phase S: scores -> first-max index encoding ----------
        idxt_tiles = {}

        def emit_score(t):
            sc_ps = psS.tile([128, 1024], f32, tag="sc", name=f"sc{t}")
            for h in range(2):
                nc.tensor.matmul(
                    sc_ps[:, h * 512 : (h + 1) * 512], or2_sb[:],
                    nc2hl_sb[:, h * 512 : (h + 1) * 512],
                    start=True, stop=False, skip_group_check=True,
                )
            for g in range(G):
                nc.tensor.matmul(
                    sc_ps[:, g * 128 : (g + 1) * 128],
                    xf_tiles[t][:, g, :], bdf_sb[:, g, :],
                    start=False, stop=(g == G - 1), skip_group_check=True,
                )
            # argmax chain on DVE (Pool rejects ALU ops / PSUM reads)
            maxb = sbm.tile([128, C], f32, tag="maxb", name=f"maxb{t}")
            nc.vector.tensor_reduce(
                maxb[:], sc_ps[:].rearrange("p (c k) -> p c k", k=KC),
                axis=X, op=AO.max,
            )
            mask = sbm.tile([128, 1024], f16, tag="mask", name=f"mask{t}")
            nc.vector.tensor_tensor(
                mask[:].rearrange("p (c k) -> p c k", k=KC),
                sc_ps[:].rearrange("p (c k) -> p c k", k=KC),
                maxb[:].rearrange("p (c u) -> p c u", u=1).broadcast_to((128, C, KC)),
                op=AO.is_equal,
            )
            # iv = mask*64 + (15-k): max picks the first (smallest-k) hit
            nc.vector.scalar_tensor_tensor(
                mask[:], mask[:], 64.0, ioneg_sb[:], op0=AO.mult, op1=AO.add
            )
            idxt = sbi.tile([128, C], f16, tag="idxt", name=f"idxt{t}")
            nc.vector.tensor_reduce(
                idxt[:], mask[:].rearrange("p (c k) -> p c k", k=KC),
                axis=X, op=AO.max,
            )
            idxt_tiles[t] = idxt

        # deferred: transpose tile t's index row into idxT (PE + DVE copy);
        # emitted a few slots after the chain so the PE never waits on it
        def emit_tp(t):
            tok = slice(t * 128, (t + 1) * 128)
            tp_ps = psT.tile([64, 128], f16, tag="tp", name=f"tp{t}")
            nc.tensor.transpose(tp_ps[:], idxt_tiles[t][:], idb_sb[:])
            nc.vector.tensor_copy(idxT_sb[:, tok], tp_ps[:])

        # one-hot expansion for (group g, token half h), straight to fp8
        def emit_oh(g, h):
            cols = slice(h * 512, (h + 1) * 512)
            idxb = sbi.tile([128, 512], f16, tag="idxb", name=f"idxb{g}_{h}")
            nc.gpsimd.dma_start(
                idxb[:],
                idxT_sb[g * 8 : (g + 1) * 8, cols]
                .rearrange("j (n u) -> j u n", u=1)
                .broadcast_to((8, KC, 512)),
            )
            nc.vector.tensor_tensor(
                oh8_sb[:, g, cols], idxb[:],
                kiota_sb[:, 0:1].broadcast_to((128, 512)),
                op=AO.is_equal,
            )

        # ---------- phase G: token-major DoubleRow gather units ----------
        # psum[tok128, 1024] += sum over 4 group-pairs x (a,b) passes; the
        # one-hot pair is the stationary operand (reused over 4 matmuls),
        # q streams 512-col chunks
        def emit_gunit(t, qtr, u):
            tok = slice(t * 128, (t + 1) * 128)
            ps = psS.tile([128, 1024], f32, tag="sc", name=f"gu{t}_{qtr}")
            for gp in range(GP):
                lhs = oh8_sb[:, 2 * gp : 2 * gp + 2, tok]
                for ch in range(2):
                    oc = slice(ch * 512, (ch + 1) * 512)
                    nc.tensor.matmul(
                        ps[:, oc], lhs, q8a_sb[:, qtr, 2 * gp : 2 * gp + 2, oc],
                        start=(gp == 0), stop=False,
                        perf_mode=DR, skip_group_check=True,
                    )
                    nc.tensor.matmul(
                        ps[:, oc], lhs, q8b_sb[:, qtr, 2 * gp : 2 * gp + 2, oc],
                        start=False, stop=(gp == GP - 1),
                        perf_mode=DR, skip_group_check=True,
                    )
            o16 = sbo.tile([128, 1024], f16, tag="o16", name=f"o16_{t}_{qtr}")
            # GPSIMD cannot read PSUM (walrus birverifier) -> DVE/Act only
            if u % 2 == 0:
                nc.vector.tensor_copy(o16[:], ps[:])
            else:
                nc.scalar.activation(o16[:], ps[:], AF.Identity, bias=0.0, scale=1.0)
            nc.sync.dma_start(out_d[tok, qtr * 1024 : (qtr + 1) * 1024], o16[:])

        # ---------- interleaved emission (PE queue is in-order!) ----------
        emit_score(0); emit_score(1); emit_score(2); emit_score(3)
        emit_tp(0)
        emit_score(4)
        emit_tp(1)
        emit_score(5)
        emit_tp(2)
        emit_score(6)
        emit_tp(3)
        for g in range(G):
            emit_oh(g, 0)
        emit_score(7)
        emit_tp(4)
        emit_gunit(0, 0, 0)
        emit_tp(5)
        emit_gunit(0, 1, 1)
        emit_tp(6)
        emit_gunit(0, 2, 2)
        emit_tp(7)
        for g in range(G):
            emit_oh(g, 1)
        u = 3
        emit_gunit(0, 3, u); u += 1
        for t in range(1, TT):
            for qtr in range(NQ):
                emit_gunit(t, qtr, u)
                u += 1

    nc.compile()
    return nc


def _consts():
    kiota = (79.0 - np.arange(128, dtype=np.float32) % KC).reshape(128, 1).astype(np.float16)
    ioneg = np.tile(
        15.0 - (np.arange(1024, dtype=np.float32) % KC), (128, 1)
    ).astype(np.float16)
    idb = np.eye(128, dtype=np.float16)
    return kiota, ioneg, idb


def _prep_inputs(x, centroids, weight, bias):
    """Host-side shard/layout prep, exact int8 fake-quant lut, fp8 split."""
    import ml_dtypes

    kiota, ioneg, idb = _consts()
    # block-diagonal centroids^T: bd[16j+s, g, 16j+k] = centroids[8g+j, k, s]
    bd = np.zeros((128, G, 128), np.float32)
    for g in range(G):
        for j in range(8):
            bd[16 * j : 16 * (j + 1), g, 16 * j : 16 * (j + 1)] = centroids[
                8 * g + j
            ].T
    nc2 = (-0.5 * (centroids.astype(np.float64) ** 2).sum(-1)).astype(
        np.float32
    ).reshape(1, C * KC)
    nc2h = nc2.astype(np.float16)
    nc2l = (nc2 - nc2h.astype(np.float32)).astype(np.float16)
    nc2hl = np.concatenate([nc2h, nc2l], axis=0)
    or2 = np.ones((2, 128), np.float16)

    # exact int8 fake-quant lut (float64, matching the oracle) + fp8 split
    lut = np.einsum(
        "cks,cso->cko", centroids.astype(np.float64), weight.astype(np.float64)
    )
    amax = np.abs(lut).max()
    scale = np.float32(amax / 127.0)
    q = np.clip(np.round(lut / (amax / 127.0)), -127.0, 127.0)
    qa16 = 16.0 * np.round(q / 16.0)   # multiples of 16, |.| <= 128
    qb = q - qa16                      # ints, |.| <= 8

    def pack(v):
        v = v.reshape(G, 8, KC, O)       # [g, j, k, o]
        v = v.transpose(1, 2, 0, 3)      # [j, k, g, o] -> p = 16j+k
        v = v.reshape(128, G, NQ, 1024)  # [p, g, qtr, oc]
        v = v.transpose(0, 2, 1, 3)      # [p, qtr, g, oc]
        return np.ascontiguousarray(v).astype(ml_dtypes.float8_e4m3)

    common = dict(
        q8a=pack(qa16), q8b=pack(qb), bdf=bd, nc2hl=nc2hl,
        or2=or2, kiota=kiota, ioneg=ioneg, idb=idb,
    )
    in_maps = []
    for i in range(NCORES):
        xs = x[i * NLOC : (i + 1) * NLOC, :]  # (1024, 1024)
        xf = np.ascontiguousarray(
            xs.T.reshape(G, 128, TT, 128).transpose(1, 2, 0, 3)
        )  # [p, t, g, n] float32
        m = dict(common)
        m.update(xf=xf)
        in_maps.append(m)
    return in_maps, scale, np.asarray(bias, np.float32)


def _assemble(per_core_outs, scale, bias32):
    out = np.concatenate(
        [np.asarray(o).astype(np.float32) for o in per_core_outs], axis=0
    )
    return out * scale + bias32[None, :]


def kernel(x, centroids, weight, inverse_temperature_logit, bias, **_):
    from concourse.bass_utils import run_bass_kernel_spmd

    x = np.asarray(x, np.float32)
    centroids = np.asarray(centroids, np.float32)
    weight = np.asarray(weight, np.float32)
    bias = np.asarray(bias, np.float32)

    if "nc" not in _CACHED:
        _CACHED["nc"] = build_nc()
    nc = _CACHED["nc"]

    in_maps, scale, bias32 = _prep_inputs(x, centroids, weight, bias)
    res = run_bass_kernel_spmd(nc, in_maps, core_ids=list(range(NCORES)))
    return _assemble(
        [res.results[i]["out"] for i in range(NCORES)], scale, bias32
    )


# revision 10
# speedup vs baseline: 1.0087x; 1.0087x over previous
"""AMMLinear (vq_codebook) forward kernel for 8 TRN2 NeuronCores.

Key algebraic fact: the reference's straight-through estimator
    output = real - stop_grad(real - quantized)
is numerically exactly `quantized_output + bias`, so the forward value needs
only:  argmin-distance one-hot  @  fake-quantized lut  + bias.

Distribution: pure data-parallel over the 8192 tokens (1024/core) with ZERO
collectives -- cores run fully independently (no barrier / AllReduce /
AllGather latency, immune to core start skew).

The quantized lut q = clip(round(lut/scale), -127, 127) is x-independent
(derived from centroids/weight only), so it is computed EXACTLY on host
(float64, matching the oracle) and shipped to the cores -- no on-device lut
matmuls or quantize epilogue at all.

The gather  out[n,o] = sum_c q[c, argmin_k dist, o]  runs as fp8e4m3
DoubleRow matmuls: q splits exactly as q = qa16 + qb with qa16 = 16*round(
q/16) (multiples of 16, |.|<=128) and qb = q - qa16 (ints, |.|<=8) -- both
exact in e4m3, as are the 0/1 one-hot weights.  DoubleRow packs TWO
codebook-groups (2 x 8 codebooks x 16 centroids = 256) into one matmul
contraction; the PE moving port is ~1KB/partition per 213ns either way, so
this matches fp16's column rate (exact int8 is 2x fp8 information) but
HALVES the weight loads, each one-hot pair reused across 4 matmuls.
The one-hot is the stationary operand (reused across all 4096 out cols),
token-major: psum[tok128, ocols] so the PSUM drain is a single dtype-convert
copy (the psum holds exact integer sums |.|<=8128; fp16 rounding of those is
<= 2^-12 relative) on the otherwise-idle Act engine, and the out DMA is
fp16, split across the sync/scalar queues.  The x-independent
out * scale + bias  epilogue runs on host in fp32.

Scores are ONE fp32 matmul pass (exact argmins; the hw runs fp32 as two
half-speed matmuls, ~as fast as the fp16 3-pass hi/lo scheme but a third
of the weight loads and no hi/lo x DMA ordering).  Score tiles are built
as HALF tiles ([128 tok, 512] psum = 1 bank, 4 codebook-groups each) so
the DVE argmax chain frees each bank after only reduce+is_equal of half a
tile -- the PE never waits on PSUM recycling.  The c2 row-pair init stays
one fp16 K=2 matmul per half (hi+lo rows summed in-psum).

Per-core pipeline: score half-tiles -> DVE argmax chain -> PE transpose ->
one-hot expand (broadcast DMA + is_equal to fp8) -> gather units (t,
o-quarter): 16 DoubleRow matmuls into a [128,1024] psum, Act convert-drain,
fp16 DMA out.  Host concatenates core shards and applies scale+bias.
"""

import numpy as np

N_TOKENS = 8192
IN_FEAT = 1024
C = 64   # codebooks
KC = 16  # centroids per codebook
S = 16   # subvector length
O = 4096  # out features
NCORES = 8
NLOC = N_TOKENS // NCORES  # 1024 tokens per core
G = 8    # groups of 8 codebooks -> 128-row contraction
GP = 4   # group-pairs (DoubleRow: 2 groups = 256-row contraction)
TT = NLOC // 128  # 8 token tiles
NQ = 4   # o-quarters of 1024 cols (one gather unit each)

_CACHED = {}


def build_nc():
    import concourse.bacc as bacc
    import concourse.mybir as mybir
    import concourse.tile as tile
    from contextlib import ExitStack

    f32 = mybir.dt.float32
    f16 = mybir.dt.float16
    f8 = mybir.dt.float8e4
    AO = mybir.AluOpType
    AF = mybir.ActivationFunctionType
    DR = mybir.MatmulPerfMode.DoubleRow
    X = mybir.AxisListType.X

    nc = bacc.Bacc(
        "TRN2", target_bir_lowering=False, debug=False, num_devices=NCORES
    )

    xf_d = nc.dram_tensor("xf", [128, TT, G, 128], f32, kind="ExternalInput")
    q8a_d = nc.dram_tensor("q8a", [128, NQ, G, 1024], f8, kind="ExternalInput")
    q8b_d = nc.dram_tensor("q8b", [128, NQ, G, 1024], f8, kind="ExternalInput")
    bdf_d = nc.dram_tensor("bdf", [128, G, 128], f32, kind="ExternalInput")
    nc2hl_d = nc.dram_tensor("nc2hl", [2, 1024], f16, kind="ExternalInput")
    or2_d = nc.dram_tensor("or2", [2, 128], f16, kind="ExternalInput")
    kiota_d = nc.dram_tensor("kiota", [128, 1], f16, kind="ExternalInput")
    ioneg_d = nc.dram_tensor("ioneg", [128, 1024], f16, kind="ExternalInput")
    idb_d = nc.dram_tensor("idb", [128, 128], f16, kind="ExternalInput")
    out_d = nc.dram_tensor("out", [NLOC, O], f16, kind="ExternalOutput")

    with ExitStack() as ctx:
        tc = ctx.enter_context(tile.TileContext(nc))
        sb = ctx.enter_context(tc.tile_pool(name="sb", bufs=1))
        # all 8 x tiles resident (4.2MB): a smaller ring spanning the two x
        # DMA queues deadlocks the tile scheduler against the psA slot ring
        sbx = ctx.enter_context(tc.tile_pool(name="sbx", bufs=8))
        sbm = ctx.enter_context(tc.tile_pool(name="sbm", bufs=3))
        sbo = ctx.enter_context(tc.tile_pool(name="sbo", bufs=8))
        sbi = ctx.enter_context(tc.tile_pool(name="sbi", bufs=4))
        psA = ctx.enter_context(tc.tile_pool(name="psA", bufs=3, space="PSUM"))
        psB = ctx.enter_context(tc.tile_pool(name="psB", bufs=2, space="PSUM"))
        psT = ctx.enter_context(tc.tile_pool(name="psT", bufs=1, space="PSUM"))

        # ---------- persistent SBUF ----------
        bdf_sb = sb.tile([128, G, 128], f32)
        nc2hl_sb = sb.tile([2, 1024], f16)
        or2_sb = sb.tile([2, 128], f16)
        kiota_sb = sb.tile([128, 1], f16)
        ioneg_sb = sb.tile([128, 1024], f16)
        idb_sb = sb.tile([128, 128], f16)
        q8a_sb = sb.tile([128, NQ, G, 1024], f8)
        q8b_sb = sb.tile([128, NQ, G, 1024], f8)
        oh8_sb = sb.tile([128, G, NLOC], f8)
        idxT_sb = sb.tile([64, NLOC], f16)

        # ---------- const + input DMAs ----------
        # small consts first so the first score matmuls aren't blocked
        nc.gpsimd.dma_start(nc2hl_sb[:], nc2hl_d[:])
        nc.gpsimd.dma_start(or2_sb[:], or2_d[:])
        nc.gpsimd.dma_start(kiota_sb[:], kiota_d[:])
        nc.gpsimd.dma_start(ioneg_sb[:], ioneg_d[:])
        nc.gpsimd.dma_start(idb_sb[:], idb_d[:])
        nc.gpsimd.dma_start(bdf_sb[:], bdf_d[:])

        # fp32 x token tiles: evens on scalar queue; odds interleaved with
        # the quantized-lut pieces on sync (q8 quarter qtr needed ~4 units in)
        xf_tiles = [
            sbx.tile([128, G, 128], f32, tag="xf", name=f"xf{t}")
            for t in range(TT)
        ]
        for t in range(0, TT, 2):
            nc.scalar.dma_start(xf_tiles[t][:], xf_d[:, t])
        sync_in = [
            ("x", 1), ("a", 0), ("x", 3), ("b", 0),
            ("x", 5), ("a", 1), ("x", 7), ("b", 1),
            ("a", 2), ("b", 2), ("a", 3), ("b", 3),
        ]
        for kind, i in sync_in:
            if kind == "x":
                nc.sync.dma_start(xf_tiles[i][:], xf_d[:, i])
            elif kind == "a":
                nc.sync.dma_start(q8a_sb[:, i], q8a_d[:, i])
            else:
                nc.sync.dma_start(q8b_sb[:, i], q8b_d[:, i])

        # ---------- phase S: scores -> first-max index encoding ----------
        idxt_tiles = {}

        def emit_score_half(t, h):
            """Half a token tile: codebook-groups 4h..4h+3 -> 1 psum bank."""
            cc = slice(h * 512, (h + 1) * 512)  # ck columns
            sc_ps = psA.tile([128, 512], f32, tag="sc", name=f"sc{t}_{h}")
            nc.tensor.matmul(
                sc_ps[:], or2_sb[:], nc2hl_sb[:, cc],
                start=True, stop=False, skip_group_check=True,
            )
            for gg in range(4):
                g = 4 * h + gg
                nc.tensor.matmul(
                    sc_ps[:, gg * 128 : (gg + 1) * 128],
                    xf_tiles[t][:, g, :], bdf_sb[:, g, :],
                    start=False, stop=(gg == 3), skip_group_check=True,
                )
            # argmax chain on DVE; psum bank freed right after is_equal
            maxb = sbm.tile([128, 32], f32, tag="maxb", name=f"maxb{t}_{h}")
            nc.vector.tensor_reduce(
                maxb[:], sc_ps[:].rearrange("p (c k) -> p c k", k=KC),
                axis=X, op=AO.max,
            )
            mask = sbm.tile([128, 512], f16, tag="mask", name=f"mask{t}_{h}")
            nc.vector.tensor_tensor(
                mask[:].rearrange("p (c k) -> p c k", k=KC),
                sc_ps[:].rearrange("p (c k) -> p c k", k=KC),
                maxb[:].rearrange("p (c u) -> p c u", u=1).broadcast_to((128, 32, KC)),
                op=AO.is_equal,
            )
            # iv = mask*64 + (15-k): max picks the first (smallest-k) hit
            nc.vector.scalar_tensor_tensor(
                mask[:], mask[:], 64.0, ioneg_sb[:, cc], op0=AO.mult, op1=AO.add
            )
            if h == 0:
                idxt_tiles[t] = sbi.tile([128, C], f16, tag="idxt", name=f"idxt{t}")
            nc.vector.tensor_reduce(
                idxt_tiles[t][:, h * 32 : (h + 1) * 32],
                mask[:].rearrange("p (c k) -> p c k", k=KC),
                axis=X, op=AO.max,
            )

        def emit_score(t):
            emit_score_half(t, 0)
            emit_score_half(t, 1)

        # deferred: transpose tile t's index row into idxT (PE + DVE copy);
        # emitted a few slots after the chain so the PE never waits on it
        def emit_tp(t):
            tok = slice(t * 128, (t + 1) * 128)
            tp_ps = psT.tile([64, 128], f16, tag="tp", name=f"tp{t}")
            nc.tensor.transpose(tp_ps[:], idxt_tiles[t][:], idb_sb[:])
            nc.vector.tensor_copy(idxT_sb[:, tok], tp_ps[:])

        # one-hot expansion for (group g, token half h), straight to fp8
        def emit_oh(g, h):
            cols = slice(h * 512, (h + 1) * 512)
            idxb = sbi.tile([128, 512], f16, tag="idxb", name=f"idxb{g}_{h}")
            nc.gpsimd.dma_start(
                idxb[:],
                idxT_sb[g * 8 : (g + 1) * 8, cols]
                .rearrange("j (n u) -> j u n", u=1)
                .broadcast_to((8, KC, 512)),
            )
            nc.vector.tensor_tensor(
                oh8_sb[:, g, cols], idxb[:],
                kiota_sb[:, 0:1].broadcast_to((128, 512)),
                op=AO.is_equal,
            )

        # ---------- phase G: token-major DoubleRow gather units ----------
        # psum[tok128, 1024] += sum over 4 group-pairs x (a,b) passes; the
        # one-hot pair is the stationary operand (reused over 4 matmuls),
        # q streams 512-col chunks
        def emit_gunit(t, qtr, u):
            tok = slice(t * 128, (t + 1) * 128)
            ps = psB.tile([128, 1024], f32, tag="gu", name=f"gu{t}_{qtr}")
            for gp in range(GP):
                lhs = oh8_sb[:, 2 * gp : 2 * gp + 2, tok]
                for ch in range(2):
                    oc = slice(ch * 512, (ch + 1) * 512)
                    nc.tensor.matmul(
                        ps[:, oc], lhs, q8a_sb[:, qtr, 2 * gp : 2 * gp + 2, oc],
                        start=(gp == 0), stop=False,
                        perf_mode=DR, skip_group_check=True,
                    )
                    nc.tensor.matmul(
                        ps[:, oc], lhs, q8b_sb[:, qtr, 2 * gp : 2 * gp + 2, oc],
                        start=False, stop=(gp == GP - 1),
                        perf_mode=DR, skip_group_check=True,
                    )
            o16 = sbo.tile([128, 1024], f16, tag="o16", name=f"o16_{t}_{qtr}")
            # drains on the otherwise-idle Act engine (GpSimd can't read
            # PSUM); the second-to-last goes to DVE so the tail overlaps
            if u == 30:
                nc.vector.tensor_copy(o16[:], ps[:])
            else:
                nc.scalar.activation(o16[:], ps[:], AF.Identity, bias=0.0, scale=1.0)
            eng = nc.sync if u % 2 == 0 else nc.scalar
            eng.dma_start(out_d[tok, qtr * 1024 : (qtr + 1) * 1024], o16[:])

        # ---------- interleaved emission (PE queue is in-order!) ----------
        # DVE queue is in-order too: the oh h0 is_equals must precede the
        # sc4..7 chains in the DVE stream or the first gather unit stalls
        emit_score(0); emit_score(1); emit_score(2); emit_score(3)
        emit_tp(0); emit_tp(1); emit_tp(2); emit_tp(3)
        for g in range(G):
            emit_oh(g, 0)
        emit_score(4)
        emit_score(5)
        emit_gunit(0, 0, 0)
        emit_score(6)
        emit_gunit(0, 1, 1)
        emit_tp(4)
        emit_score(7)
        emit_gunit(0, 2, 2)
        emit_tp(5)
        emit_gunit(0, 3, 3)
        emit_tp(6); emit_tp(7)
        for g in range(G):
            emit_oh(g, 1)
        u = 4
        for t in range(1, TT):
            for qtr in range(NQ):
                emit_gunit(t, qtr, u)
                u += 1

    nc.compile()
    return nc


def _consts():
    kiota = (79.0 - np.arange(128, dtype=np.float32) % KC).reshape(128, 1).astype(np.float16)
    ioneg = np.tile(
        15.0 - (np.arange(1024, dtype=np.float32) % KC), (128, 1)
    ).astype(np.float16)
    idb = np.eye(128, dtype=np.float16)
    return kiota, ioneg, idb


def _prep_inputs(x, centroids, weight, bias):
    """Host-side shard/layout prep, exact int8 fake-quant lut, fp8 split."""
    import ml_dtypes

    kiota, ioneg, idb = _consts()
    # block-diagonal centroids^T: bd[16j+s, g, 16j+k] = centroids[8g+j, k, s]
    bd = np.zeros((128, G, 128), np.float32)
    for g in range(G):
        for j in range(8):
            bd[16 * j : 16 * (j + 1), g, 16 * j : 16 * (j + 1)] = centroids[
                8 * g + j
            ].T
    nc2 = (-0.5 * (centroids.astype(np.float64) ** 2).sum(-1)).astype(
        np.float32
    ).reshape(1, C * KC)
    nc2h = nc2.astype(np.float16)
    nc2l = (nc2 - nc2h.astype(np.float32)).astype(np.float16)
    nc2hl = np.concatenate([nc2h, nc2l], axis=0)
    or2 = np.ones((2, 128), np.float16)

    # exact int8 fake-quant lut (float64, matching the oracle) + fp8 split
    lut = np.einsum(
        "cks,cso->cko", centroids.astype(np.float64), weight.astype(np.float64)
    )
    amax = np.abs(lut).max()
    scale = np.float32(amax / 127.0)
    q = np.clip(np.round(lut / (amax / 127.0)), -127.0, 127.0)
    qa16 = 16.0 * np.round(q / 16.0)   # multiples of 16, |.| <= 128
    qb = q - qa16                      # ints, |.| <= 8

    def pack(v):
        v = v.reshape(G, 8, KC, O)       # [g, j, k, o]
        v = v.transpose(1, 2, 0, 3)      # [j, k, g, o] -> p = 16j+k
        v = v.reshape(128, G, NQ, 1024)  # [p, g, qtr, oc]
        v = v.transpose(0, 2, 1, 3)      # [p, qtr, g, oc]
        return np.ascontiguousarray(v).astype(ml_dtypes.float8_e4m3)

    common = dict(
        q8a=pack(qa16), q8b=pack(qb), bdf=bd, nc2hl=nc2hl,
        or2=or2, kiota=kiota, ioneg=ioneg, idb=idb,
    )
    in_maps = []
    for i in range(NCORES):
        xs = x[i * NLOC : (i + 1) * NLOC, :]  # (1024, 1024)
        xf = np.ascontiguousarray(
            xs.T.reshape(G, 128, TT, 128).transpose(1, 2, 0, 3)
        )  # [p, t, g, n] float32
        m = dict(common)
        m.update(xf=xf)
        in_maps.append(m)
    return in_maps, scale, np.asarray(bias, np.float32)


def _assemble(per_core_outs, scale, bias32):
    out = np.concatenate(
        [np.asarray(o).astype(np.float32) for o in per_core_outs], axis=0
    )
    return out * scale + bias32[None, :]


def kernel(x, centroids, weight, inverse_temperature_logit, bias, **_):
    from concourse.bass_utils import run_bass_kernel_spmd

    x = np.asarray(x, np.float32)
    centroids = np.asarray(centroids, np.float32)
    weight = np.asarray(weight, np.float32)
    bias = np.asarray(bias, np.float32)

    if "nc" not in _CACHED:
        _CACHED["nc"] = build_nc()
    nc = _CACHED["nc"]

    in_maps, scale, bias32 = _prep_inputs(x, centroids, weight, bias)
    res = run_bass_kernel_spmd(nc, in_maps, core_ids=list(range(NCORES)))
    return _assemble(
        [res.results[i]["out"] for i in range(NCORES)], scale, bias32
    )


# revision 11
# speedup vs baseline: 1.0255x; 1.0167x over previous
"""AMMLinear (vq_codebook) forward kernel for 8 TRN2 NeuronCores.

Key algebraic fact: the reference's straight-through estimator
    output = real - stop_grad(real - quantized)
is numerically exactly `quantized_output + bias`, so the forward value needs
only:  argmin-distance one-hot  @  fake-quantized lut  + bias.

Distribution: pure data-parallel over the 8192 tokens (1024/core) with ZERO
collectives -- cores run fully independently (no barrier / AllReduce /
AllGather latency, immune to core start skew).

The quantized lut q = clip(round(lut/scale), -127, 127) is x-independent
(derived from centroids/weight only), so it is computed EXACTLY on host
(float64, matching the oracle) and shipped to the cores -- no on-device lut
matmuls or quantize epilogue at all.

The gather  out[n,o] = sum_c q[c, argmin_k dist, o]  runs as fp8e4m3
DoubleRow matmuls: q splits exactly as q = qa16 + qb with qa16 = 16*round(
q/16) (multiples of 16, |.|<=128) and qb = q - qa16 (ints, |.|<=8) -- both
exact in e4m3, as are the 0/1 one-hot weights.  DoubleRow packs TWO
codebook-groups (2 x 8 codebooks x 16 centroids = 256) into one matmul
contraction; the PE moving port is ~1KB/partition per 213ns either way, so
this matches fp16's column rate (exact int8 is 2x fp8 information) but
HALVES the weight loads, each one-hot pair reused across 4 matmuls.
The one-hot is the stationary operand (reused across all 4096 out cols),
token-major: psum[tok128, ocols] so the PSUM drain is a single dtype-convert
copy (the psum holds exact integer sums |.|<=8128; fp16 rounding of those is
<= 2^-12 relative) on the otherwise-idle Act engine, and the out DMA is
fp16, split across the sync/scalar queues.  The x-independent
out * scale + bias  epilogue runs on host in fp32.

Scores are ONE fp32 matmul pass (exact argmins; the hw runs fp32 as two
half-speed matmuls, ~as fast as the fp16 3-pass hi/lo scheme but a third
of the weight loads and no hi/lo x DMA ordering).  Score tiles are built
as HALF tiles ([128 tok, 512] psum = 1 bank, 4 codebook-groups each) so
the DVE argmax chain frees each bank after only reduce+is_equal of half a
tile -- the PE never waits on PSUM recycling.  The c2 row-pair init stays
one fp16 K=2 matmul per half (hi+lo rows summed in-psum).

Per-core pipeline: score half-tiles -> DVE argmax chain -> PE transpose ->
one-hot expand (broadcast DMA + is_equal to fp8) -> gather units (t,
o-quarter): 16 DoubleRow matmuls into a [128,1024] psum, Act convert-drain,
fp16 DMA out.  Host concatenates core shards and applies scale+bias.
"""

import numpy as np

N_TOKENS = 8192
IN_FEAT = 1024
C = 64   # codebooks
KC = 16  # centroids per codebook
S = 16   # subvector length
O = 4096  # out features
NCORES = 8
NLOC = N_TOKENS // NCORES  # 1024 tokens per core
G = 8    # groups of 8 codebooks -> 128-row contraction
GP = 4   # group-pairs (DoubleRow: 2 groups = 256-row contraction)
TT = NLOC // 128  # 8 token tiles
NQ = 4   # o-quarters of 1024 cols (one gather unit each)

_CACHED = {}


def build_nc():
    import concourse.bacc as bacc
    import concourse.mybir as mybir
    import concourse.tile as tile
    from contextlib import ExitStack

    f32 = mybir.dt.float32
    f16 = mybir.dt.float16
    f8 = mybir.dt.float8e4
    AO = mybir.AluOpType
    AF = mybir.ActivationFunctionType
    DR = mybir.MatmulPerfMode.DoubleRow
    X = mybir.AxisListType.X

    nc = bacc.Bacc(
        "TRN2", target_bir_lowering=False, debug=False, num_devices=NCORES
    )

    xf_d = nc.dram_tensor("xf", [128, TT, G, 128], f32, kind="ExternalInput")
    q8a_d = nc.dram_tensor("q8a", [128, NQ, G, 1024], f8, kind="ExternalInput")
    q8b_d = nc.dram_tensor("q8b", [128, NQ, G, 1024], f8, kind="ExternalInput")
    bdf_d = nc.dram_tensor("bdf", [128, G, 128], f32, kind="ExternalInput")
    nc2hl_d = nc.dram_tensor("nc2hl", [2, 1024], f16, kind="ExternalInput")
    or2_d = nc.dram_tensor("or2", [2, 128], f16, kind="ExternalInput")
    kiota_d = nc.dram_tensor("kiota", [128, 1], f16, kind="ExternalInput")
    ioneg_d = nc.dram_tensor("ioneg", [128, 1024], f16, kind="ExternalInput")
    idb_d = nc.dram_tensor("idb", [128, 128], f16, kind="ExternalInput")
    out_d = nc.dram_tensor("out", [NLOC, O], f16, kind="ExternalOutput")

    with ExitStack() as ctx:
        tc = ctx.enter_context(tile.TileContext(nc))
        sb = ctx.enter_context(tc.tile_pool(name="sb", bufs=1))
        # all 8 x tiles resident (4.2MB): a smaller ring spanning the two x
        # DMA queues deadlocks the tile scheduler against the psA slot ring
        sbx = ctx.enter_context(tc.tile_pool(name="sbx", bufs=8))
        sbm = ctx.enter_context(tc.tile_pool(name="sbm", bufs=3))
        sbo = ctx.enter_context(tc.tile_pool(name="sbo", bufs=8))
        sbi = ctx.enter_context(tc.tile_pool(name="sbi", bufs=4))
        psA = ctx.enter_context(tc.tile_pool(name="psA", bufs=3, space="PSUM"))
        psB = ctx.enter_context(tc.tile_pool(name="psB", bufs=2, space="PSUM"))
        psT = ctx.enter_context(tc.tile_pool(name="psT", bufs=1, space="PSUM"))

        # ---------- persistent SBUF ----------
        bdf_sb = sb.tile([128, G, 128], f32)
        nc2hl_sb = sb.tile([2, 1024], f16)
        or2_sb = sb.tile([2, 128], f16)
        kiota_sb = sb.tile([128, 1], f16)
        ioneg_sb = sb.tile([128, 1024], f16)
        idb_sb = sb.tile([128, 128], f16)
        q8a_sb = sb.tile([128, NQ, G, 1024], f8)
        q8b_sb = sb.tile([128, NQ, G, 1024], f8)
        oh8_sb = sb.tile([128, G, NLOC], f8)
        idxT_sb = sb.tile([64, NLOC], f16)

        # ---------- const + input DMAs ----------
        # ALL queues share ~350GB/s of per-core DMA bandwidth, so global
        # need-order is what matters: score-phase operands (or2/nc2hl/bdf +
        # x tiles) stream strictly BEFORE the 8.4MB quantized lut, which is
        # first needed ~30us in.
        nc.gpsimd.dma_start(or2_sb[:], or2_d[:])
        nc.gpsimd.dma_start(nc2hl_sb[:], nc2hl_d[:])
        nc.gpsimd.dma_start(bdf_sb[:], bdf_d[:])
        nc.gpsimd.dma_start(kiota_sb[:], kiota_d[:])
        nc.gpsimd.dma_start(idb_sb[:], idb_d[:])
        nc.gpsimd.dma_start(ioneg_sb[:], ioneg_d[:])

        xf_tiles = [
            sbx.tile([128, G, 128], f32, tag="xf", name=f"xf{t}")
            for t in range(TT)
        ]
        for t in range(0, TT, 2):
            nc.scalar.dma_start(xf_tiles[t][:], xf_d[:, t])
        for t in range(1, TT, 2):
            nc.sync.dma_start(xf_tiles[t][:], xf_d[:, t])
        for qtr in range(NQ):
            nc.sync.dma_start(q8a_sb[:, qtr], q8a_d[:, qtr])
            nc.sync.dma_start(q8b_sb[:, qtr], q8b_d[:, qtr])

        # ---------- phase S: scores -> first-max index encoding ----------
        idxt_tiles = {}

        def emit_score_half(t, h):
            """Half a token tile: codebook-groups 4h..4h+3 -> 1 psum bank."""
            cc = slice(h * 512, (h + 1) * 512)  # ck columns
            sc_ps = psA.tile([128, 512], f32, tag="sc", name=f"sc{t}_{h}")
            nc.tensor.matmul(
                sc_ps[:], or2_sb[:], nc2hl_sb[:, cc],
                start=True, stop=False, skip_group_check=True,
            )
            for gg in range(4):
                g = 4 * h + gg
                nc.tensor.matmul(
                    sc_ps[:, gg * 128 : (gg + 1) * 128],
                    xf_tiles[t][:, g, :], bdf_sb[:, g, :],
                    start=False, stop=(gg == 3), skip_group_check=True,
                )
            # argmax chain on DVE; psum bank freed right after is_equal
            maxb = sbm.tile([128, 32], f32, tag="maxb", name=f"maxb{t}_{h}")
            nc.vector.tensor_reduce(
                maxb[:], sc_ps[:].rearrange("p (c k) -> p c k", k=KC),
                axis=X, op=AO.max,
            )
            mask = sbm.tile([128, 512], f16, tag="mask", name=f"mask{t}_{h}")
            nc.vector.tensor_tensor(
                mask[:].rearrange("p (c k) -> p c k", k=KC),
                sc_ps[:].rearrange("p (c k) -> p c k", k=KC),
                maxb[:].rearrange("p (c u) -> p c u", u=1).broadcast_to((128, 32, KC)),
                op=AO.is_equal,
            )
            # iv = mask*64 + (15-k): max picks the first (smallest-k) hit
            nc.vector.scalar_tensor_tensor(
                mask[:], mask[:], 64.0, ioneg_sb[:, cc], op0=AO.mult, op1=AO.add
            )
            if h == 0:
                idxt_tiles[t] = sbi.tile([128, C], f16, tag="idxt", name=f"idxt{t}")
            nc.vector.tensor_reduce(
                idxt_tiles[t][:, h * 32 : (h + 1) * 32],
                mask[:].rearrange("p (c k) -> p c k", k=KC),
                axis=X, op=AO.max,
            )

        def emit_score(t):
            emit_score_half(t, 0)
            emit_score_half(t, 1)

        # deferred: transpose tile t's index row into idxT (PE + DVE copy);
        # emitted a few slots after the chain so the PE never waits on it
        def emit_tp(t):
            tok = slice(t * 128, (t + 1) * 128)
            tp_ps = psT.tile([64, 128], f16, tag="tp", name=f"tp{t}")
            nc.tensor.transpose(tp_ps[:], idxt_tiles[t][:], idb_sb[:])
            nc.vector.tensor_copy(idxT_sb[:, tok], tp_ps[:])

        # one-hot expansion for (group g, token half h), straight to fp8
        def emit_oh(g, h):
            cols = slice(h * 512, (h + 1) * 512)
            idxb = sbi.tile([128, 512], f16, tag="idxb", name=f"idxb{g}_{h}")
            nc.gpsimd.dma_start(
                idxb[:],
                idxT_sb[g * 8 : (g + 1) * 8, cols]
                .rearrange("j (n u) -> j u n", u=1)
                .broadcast_to((8, KC, 512)),
            )
            nc.vector.tensor_tensor(
                oh8_sb[:, g, cols], idxb[:],
                kiota_sb[:, 0:1].broadcast_to((128, 512)),
                op=AO.is_equal,
            )

        # ---------- phase G: token-major DoubleRow gather units ----------
        # psum[tok128, 1024] += sum over 4 group-pairs x (a,b) passes; the
        # one-hot pair is the stationary operand (reused over 4 matmuls),
        # q streams 512-col chunks
        def emit_gunit(t, qtr, u):
            tok = slice(t * 128, (t + 1) * 128)
            ps = psB.tile([128, 1024], f32, tag="gu", name=f"gu{t}_{qtr}")
            for gp in range(GP):
                lhs = oh8_sb[:, 2 * gp : 2 * gp + 2, tok]
                for ch in range(2):
                    oc = slice(ch * 512, (ch + 1) * 512)
                    nc.tensor.matmul(
                        ps[:, oc], lhs, q8a_sb[:, qtr, 2 * gp : 2 * gp + 2, oc],
                        start=(gp == 0), stop=False,
                        perf_mode=DR, skip_group_check=True,
                    )
                    nc.tensor.matmul(
                        ps[:, oc], lhs, q8b_sb[:, qtr, 2 * gp : 2 * gp + 2, oc],
                        start=False, stop=(gp == GP - 1),
                        perf_mode=DR, skip_group_check=True,
                    )
            o16 = sbo.tile([128, 1024], f16, tag="o16", name=f"o16_{t}_{qtr}")
            # drains on the otherwise-idle Act engine (GpSimd can't read
            # PSUM); the second-to-last goes to DVE so the tail overlaps
            if u == 30:
                nc.vector.tensor_copy(o16[:], ps[:])
            else:
                nc.scalar.activation(o16[:], ps[:], AF.Identity, bias=0.0, scale=1.0)
            eng = nc.sync if u % 2 == 0 else nc.scalar
            eng.dma_start(out_d[tok, qtr * 1024 : (qtr + 1) * 1024], o16[:])

        # ---------- interleaved emission (PE queue is in-order!) ----------
        # DVE queue is in-order too: the oh h0 is_equals must precede the
        # sc4..7 chains in the DVE stream or the first gather unit stalls
        emit_score(0); emit_score(1); emit_score(2); emit_score(3)
        emit_tp(0); emit_tp(1); emit_tp(2); emit_tp(3)
        for g in range(G):
            emit_oh(g, 0)
        emit_score(4)
        emit_score(5)
        emit_gunit(0, 0, 0)
        emit_score(6)
        emit_gunit(0, 1, 1)
        emit_tp(4)
        emit_score(7)
        emit_gunit(0, 2, 2)
        emit_tp(5)
        emit_gunit(0, 3, 3)
        emit_tp(6); emit_tp(7)
        for g in range(G):
            emit_oh(g, 1)
        u = 4
        for t in range(1, TT):
            for qtr in range(NQ):
                emit_gunit(t, qtr, u)
                u += 1

    nc.compile()
    return nc


def _consts():
    kiota = (79.0 - np.arange(128, dtype=np.float32) % KC).reshape(128, 1).astype(np.float16)
    ioneg = np.tile(
        15.0 - (np.arange(1024, dtype=np.float32) % KC), (128, 1)
    ).astype(np.float16)
    idb = np.eye(128, dtype=np.float16)
    return kiota, ioneg, idb


def _prep_inputs(x, centroids, weight, bias):
    """Host-side shard/layout prep, exact int8 fake-quant lut, fp8 split."""
    import ml_dtypes

    kiota, ioneg, idb = _consts()
    # block-diagonal centroids^T: bd[16j+s, g, 16j+k] = centroids[8g+j, k, s]
    bd = np.zeros((128, G, 128), np.float32)
    for g in range(G):
        for j in range(8):
            bd[16 * j : 16 * (j + 1), g, 16 * j : 16 * (j + 1)] = centroids[
                8 * g + j
            ].T
    nc2 = (-0.5 * (centroids.astype(np.float64) ** 2).sum(-1)).astype(
        np.float32
    ).reshape(1, C * KC)
    nc2h = nc2.astype(np.float16)
    nc2l = (nc2 - nc2h.astype(np.float32)).astype(np.float16)
    nc2hl = np.concatenate([nc2h, nc2l], axis=0)
    or2 = np.ones((2, 128), np.float16)

    # exact int8 fake-quant lut (float64, matching the oracle) + fp8 split
    lut = np.einsum(
        "cks,cso->cko", centroids.astype(np.float64), weight.astype(np.float64)
    )
    amax = np.abs(lut).max()
    scale = np.float32(amax / 127.0)
    q = np.clip(np.round(lut / (amax / 127.0)), -127.0, 127.0)
    qa16 = 16.0 * np.round(q / 16.0)   # multiples of 16, |.| <= 128
    qb = q - qa16                      # ints, |.| <= 8

    def pack(v):
        v = v.reshape(G, 8, KC, O)       # [g, j, k, o]
        v = v.transpose(1, 2, 0, 3)      # [j, k, g, o] -> p = 16j+k
        v = v.reshape(128, G, NQ, 1024)  # [p, g, qtr, oc]
        v = v.transpose(0, 2, 1, 3)      # [p, qtr, g, oc]
        return np.ascontiguousarray(v).astype(ml_dtypes.float8_e4m3)

    common = dict(
        q8a=pack(qa16), q8b=pack(qb), bdf=bd, nc2hl=nc2hl,
        or2=or2, kiota=kiota, ioneg=ioneg, idb=idb,
    )
    in_maps = []
    for i in range(NCORES):
        xs = x[i * NLOC : (i + 1) * NLOC, :]  # (1024, 1024)
        xf = np.ascontiguousarray(
            xs.T.reshape(G, 128, TT, 128).transpose(1, 2, 0, 3)
        )  # [p, t, g, n] float32
        m = dict(common)
        m.update(xf=xf)
        in_maps.append(m)
    return in_maps, scale, np.asarray(bias, np.float32)


def _assemble(per_core_outs, scale, bias32):
    out = np.concatenate(
        [np.asarray(o).astype(np.float32) for o in per_core_outs], axis=0
    )
    return out * scale + bias32[None, :]


def kernel(x, centroids, weight, inverse_temperature_logit, bias, **_):
    from concourse.bass_utils import run_bass_kernel_spmd

    x = np.asarray(x, np.float32)
    centroids = np.asarray(centroids, np.float32)
    weight = np.asarray(weight, np.float32)
    bias = np.asarray(bias, np.float32)

    if "nc" not in _CACHED:
        _CACHED["nc"] = build_nc()
    nc = _CACHED["nc"]

    in_maps, scale, bias32 = _prep_inputs(x, centroids, weight, bias)
    res = run_bass_kernel_spmd(nc, in_maps, core_ids=list(range(NCORES)))
    return _assemble(
        [res.results[i]["out"] for i in range(NCORES)], scale, bias32
    )
